# revision 4
# baseline (speedup 1.0000x reference)
"""MoELoRA forward on 8 Trainium2 NeuronCores.

Strategy: data-parallel over tokens (B*S = 4096 -> 512 tokens/core), weights
replicated. All device matmuls run in fp32r (full PE rate). Host-side prep
pre-transposes operands to feature-major so the contraction dim sits on SBUF
partitions and no on-chip transposes are needed.

Per-core math (tokens n = 512, h = o = 1024, er = E*R = 128):
  glT[er, n] = sum_h gwrepT[h, er] * xT[h, n]     (logits, replicated 16x over er)
  E = exp(glT);  s[n] = sum_er E[er, n] = 16 * sum_e exp(logit)
  B[er, n] = 1/s[n]  (broadcast via K=1 matmul)
  gp = E * B                 == scaling * gate[e, n] (scaling=1/16 cancels the 16)
  tT[er, n] = sum_h u2T[h, er] * xT[h, n]
  tp = tT * gp
  out[n, o] = sum_h xT[h, n]*wT[h, o]  +  sum_er tp[er, n]*v2[er, o]
"""

import numpy as np

_CACHE = {}

B, S, D_IN, D_OUT, E, R = 4, 1024, 1024, 1024, 8, 16
N_CORES = 8
N_TOK = B * S                 # 4096
TOK_PER_CORE = N_TOK // N_CORES   # 512
ER = E * R                    # 128
HB = D_IN // 128              # 8 h-blocks
NB = TOK_PER_CORE // 128      # 4 token blocks
OC = D_OUT // 512             # 2 output column chunks


def _build():
    import concourse.tile as tile
    from concourse import bacc, mybir
    from contextlib import ExitStack

    f32 = mybir.dt.float32
    f32r = mybir.dt.float32r
    Exp = mybir.ActivationFunctionType.Exp

    nc = bacc.Bacc("TRN2", target_bir_lowering=False, debug=False,
                   num_devices=N_CORES)
    xT_d = nc.dram_tensor("xT", [D_IN, TOK_PER_CORE], f32r, kind="ExternalInput").ap()
    wT_d = nc.dram_tensor("wT", [D_IN, D_OUT], f32r, kind="ExternalInput").ap()
    u2T_d = nc.dram_tensor("u2T", [D_IN, ER], f32r, kind="ExternalInput").ap()
    gwT_d = nc.dram_tensor("gwT", [D_IN, ER], f32r, kind="ExternalInput").ap()
    v2_d = nc.dram_tensor("v2", [ER, D_OUT], f32r, kind="ExternalInput").ap()
    out_d = nc.dram_tensor("out", [TOK_PER_CORE, D_OUT], f32, kind="ExternalOutput").ap()

    with tile.TileContext(nc) as tc, ExitStack() as ctx:
        sb = ctx.enter_context(tc.tile_pool(name="sb", bufs=1))
        ps = ctx.enter_context(tc.tile_pool(name="ps", bufs=8, space="PSUM"))

        xT = sb.tile([128, HB, TOK_PER_CORE], f32r, tag="xT")
        wT = sb.tile([128, HB, D_OUT], f32r, tag="wT")
        u2T = sb.tile([128, HB, ER], f32r, tag="u2T")
        gwT = sb.tile([128, HB, ER], f32r, tag="gwT")
        v2 = sb.tile([128, D_OUT], f32r, tag="v2")
        ones_col = sb.tile([128, 1], f32, tag="ones_col")
        ones_row = sb.tile([1, 128], f32, tag="ones_row")

        # ring A (scalar HWDGE): gating/lora operands + xT, then output later
        nc.scalar.dma_start(gwT[:], gwT_d.rearrange("(hb p) e -> p hb e", p=128))
        nc.scalar.dma_start(u2T[:], u2T_d.rearrange("(hb p) e -> p hb e", p=128))
        nc.scalar.dma_start(v2[:], v2_d[:])
        for hb in range(HB):
            nc.scalar.dma_start(xT[:, hb, :], xT_d[hb * 128:(hb + 1) * 128, :])
        # ring B (sync HWDGE): the big weight stream, grouped by output half so
        # the first half of the outputs can finish while the second streams
        for oc in range(OC):
            for hb in range(HB):
                nc.sync.dma_start(wT[:, hb, oc * 512:(oc + 1) * 512],
                                  wT_d[hb * 128:(hb + 1) * 128, oc * 512:(oc + 1) * 512])
        nc.vector.memset(ones_col[:], 1.0)
        nc.vector.memset(ones_row[:], 1.0)

        # gate logits (replicated 16x over partitions) and tT
        gl_ps = ps.tile([128, TOK_PER_CORE], f32, tag="ps")
        for hb in range(HB):
            nc.tensor.matmul(gl_ps[:], gwT[:, hb, :], xT[:, hb, :],
                             start=(hb == 0), stop=(hb == HB - 1))
        t_ps = ps.tile([128, TOK_PER_CORE], f32, tag="ps")
        for hb in range(HB):
            nc.tensor.matmul(t_ps[:], u2T[:, hb, :], xT[:, hb, :],
                             start=(hb == 0), stop=(hb == HB - 1))

        # softmax pieces: E = exp(logits); s = colsum(E); B = bcast(1/s)
        e_sb = sb.tile([128, TOK_PER_CORE], f32, tag="e_sb")
        nc.scalar.activation(e_sb[:], gl_ps[:], Exp)
        # (the two tiny aux matmuls run in plain f32 — f32r would require
        # their on-chip-produced operands to be f32r-rounded by the producer)
        s_ps = ps.tile([1, TOK_PER_CORE], f32, tag="ps")
        nc.tensor.matmul(s_ps[:], ones_col[:], e_sb[:], start=True, stop=True)
        r_sb = sb.tile([1, TOK_PER_CORE], f32, tag="r_sb")
        nc.vector.reciprocal(r_sb[:], s_ps[:])
        b_ps = ps.tile([128, TOK_PER_CORE], f32, tag="ps")
        nc.tensor.matmul(b_ps[:], ones_row[:], r_sb[:], start=True, stop=True)

        # gp = E * B  (== scaling*gate, replicated over r); tp = tT * gp
        gp_sb = sb.tile([128, TOK_PER_CORE], f32, tag="gp_sb")
        nc.vector.tensor_mul(gp_sb[:], e_sb[:], b_ps[:])
        tp_sb = sb.tile([128, TOK_PER_CORE], f32r, tag="tp_sb")
        nc.vector.tensor_mul(tp_sb[:], t_ps[:], gp_sb[:])

        # fused base + lora accumulation, hb-outer so PE tracks the wT stream
        accs = [ps.tile([128, 512], f32, tag="ps", name=f"acc{i}")
                for i in range(NB * OC)]
        osb = ctx.enter_context(tc.tile_pool(name="osb", bufs=4))

        def acc_mm(i, hb, start, stop):
            nb, oc = divmod(i, OC)
            nc.tensor.matmul(accs[i][:],
                             xT[:, hb, nb * 128:(nb + 1) * 128],
                             wT[:, hb, oc * 512:(oc + 1) * 512],
                             start=start, stop=stop, skip_group_check=True)

        for hb in range(3):
            for i in range(NB * OC):
                acc_mm(i, hb, hb == 0, False)
        # lora term folded mid-stream (tp is ready by now)
        for i in range(NB * OC):
            nb, oc = divmod(i, OC)
            nc.tensor.matmul(accs[i][:],
                             tp_sb[:, nb * 128:(nb + 1) * 128],
                             v2[:, oc * 512:(oc + 1) * 512],
                             start=False, stop=False, skip_group_check=True)
        for hb in range(3, HB):
            for i in range(NB * OC):
                acc_mm(i, hb, False, hb == HB - 1)

        # evacuate + store, one (nb, oc) half-tile at a time for early writes
        for oc in range(OC):
            for nb in range(NB):
                i = nb * OC + oc
                o_sb = osb.tile([128, 512], f32, tag="o")
                nc.vector.tensor_copy(o_sb[:], accs[i][:])
                nc.scalar.dma_start(
                    out_d[nb * 128:(nb + 1) * 128, oc * 512:(oc + 1) * 512], o_sb[:])

    nc.compile()
    return nc


def _get_nc():
    if "nc" not in _CACHE:
        _CACHE["nc"] = _build()
    return _CACHE["nc"]


def _prep_in_maps(x, weight, gate_w, lora_U, lora_V):
    xt = np.ascontiguousarray(x.reshape(N_TOK, D_IN).T)          # (D_IN, N_TOK)
    wT = np.ascontiguousarray(weight.T)                          # (D_IN, D_OUT)
    u2T = np.ascontiguousarray(lora_U.reshape(ER, D_IN).T)       # (D_IN, ER)
    gwT = np.ascontiguousarray(np.repeat(gate_w, R, axis=0).T)   # (D_IN, ER)
    v2 = np.ascontiguousarray(lora_V.transpose(0, 2, 1).reshape(ER, D_OUT))
    in_maps = []
    for c in range(N_CORES):
        in_maps.append({
            "xT": np.ascontiguousarray(xt[:, c * TOK_PER_CORE:(c + 1) * TOK_PER_CORE]),
            "wT": wT,
            "u2T": u2T,
            "gwT": gwT,
            "v2": v2,
        })
    return in_maps


def kernel(x, weight, gate_w, lora_U, lora_V):
    from concourse import bass_utils

    x = np.asarray(x, dtype=np.float32)
    weight = np.asarray(weight, dtype=np.float32)
    gate_w = np.asarray(gate_w, dtype=np.float32)
    lora_U = np.asarray(lora_U, dtype=np.float32)
    lora_V = np.asarray(lora_V, dtype=np.float32)

    nc = _get_nc()
    in_maps = _prep_in_maps(x, weight, gate_w, lora_U, lora_V)
    res = bass_utils.run_bass_kernel_spmd(nc, in_maps, core_ids=list(range(N_CORES)))
    out = np.concatenate([res.results[c]["out"] for c in range(N_CORES)], axis=0)
    return out.reshape(B, S, D_OUT)


# revision 39
# speedup vs baseline: 1.5521x; 1.5521x over previous
"""MoELoRA forward on 8 Trainium2 NeuronCores.

Strategy: data-parallel over tokens (B*S = 4096 -> 512 tokens/core), weights
replicated. All big matmuls run in fp32r (full PE rate, ~1e-4 rel err).
Host-side prep pre-transposes operands to feature-major so the contraction dim
sits on SBUF partitions and no on-chip transposes are needed.

Per-core math (tokens n = 512, h = o = 1024, er = E*R = 128):
  gl[e, n] = sum_h gwT[h, e] * xT[h, n]            (gate logits, PE)
  Eexp = exp(gl)                                   (ACT)
  s[n]  = sum_e Eexp[e, n]                         (POOL partition_all_reduce)
  gate8 = Eexp * (1/s)                             (DVE reciprocal + mult)
  gp[er, n] = (SEL/R).T @ gate8 = gate[er//16, n]/R  (PE K=8 expand matmul;
                                                    lora scaling 1/R folded in)
  tT[er, n] = sum_h u2T[h, er] * xT[h, n]          (PE)
  tp = tT * gp                                     (DVE)
  out[n, o] = sum_h xT[h, n]*wT[h, o] + sum_er tp[er, n]*v2[er, o]
     (base and lora accumulate into the same PSUM banks; softmax normalization
      makes the base gate-weighting sum to 1, so base needs no gating)

Scheduling: input DMAs ride the sync (SP) HWDGE ring in exact consumption
order; outputs ride the scalar (ACT) ring. The PE instruction stream is
ordered so each matmul's operands have already streamed in by the time the
in-order engine reaches it.
"""

import numpy as np

_CACHE = {}

B, S, D_IN, D_OUT, E, R = 4, 1024, 1024, 1024, 8, 16
N_CORES = 8
N_TOK = B * S                 # 4096
TOK_PER_CORE = N_TOK // N_CORES   # 512
ER = E * R                    # 128
HB = D_IN // 128              # 8 h-blocks
NB = TOK_PER_CORE // 128      # 4 token blocks
OC = D_OUT // 512             # 2 output column chunks


def _build():
    import concourse.tile as tile
    import concourse.bass_isa as bass_isa
    from concourse import bacc, mybir
    from contextlib import ExitStack

    f32 = mybir.dt.float32
    f32r = mybir.dt.float32r
    Exp = mybir.ActivationFunctionType.Exp

    nc = bacc.Bacc("TRN2", target_bir_lowering=False, debug=False,
                   num_devices=N_CORES)
    xT_d = nc.dram_tensor("xT", [D_IN, TOK_PER_CORE], f32r, kind="ExternalInput").ap()
    wT_d = nc.dram_tensor("wT", [D_IN, D_OUT], f32r, kind="ExternalInput").ap()
    u2T_d = nc.dram_tensor("u2T", [D_IN, ER], f32r, kind="ExternalInput").ap()
    gwT_d = nc.dram_tensor("gwT", [D_IN, E], f32r, kind="ExternalInput").ap()
    v2_d = nc.dram_tensor("v2", [ER, D_OUT], f32r, kind="ExternalInput").ap()
    consts_d = nc.dram_tensor("consts", [E, 768], f32r, kind="ExternalInput").ap()
    out_d = nc.dram_tensor("out", [TOK_PER_CORE, D_OUT], f32, kind="ExternalOutput").ap()

    with tile.TileContext(nc) as tc, ExitStack() as ctx:
        sb = ctx.enter_context(tc.tile_pool(name="sb", bufs=1))
        ps = ctx.enter_context(tc.tile_pool(name="ps", bufs=8, space="PSUM"))

        xT = sb.tile([128, HB, TOK_PER_CORE], f32r, tag="xT")
        wT = sb.tile([128, HB, D_OUT], f32r, tag="wT")
        u2T = sb.tile([128, HB, ER], f32r, tag="u2T")
        gwT = sb.tile([128, HB, E], f32r, tag="gwT")
        v2 = sb.tile([128, D_OUT], f32r, tag="v2")
        consts = sb.tile([8, 768], f32r, tag="consts")
        ones_col = consts[:, 0:1]
        ones_row = consts[0:1, 1:9]
        sel = consts[:, 9:9 + ER]
        warm_sb = consts[0:1, 256:768]

        # input DMAs on the sync/SP ring, in consumption order
        nc.sync.dma_start(consts[:], consts_d[:])
        nc.sync.dma_start(gwT[:], gwT_d.rearrange("(hb p) e -> p hb e", p=128))
        for hb in range(HB):
            nc.sync.dma_start(xT[:, hb, :], xT_d[hb * 128:(hb + 1) * 128, :])
        # u2T rides the scalar ring so it doesn't displace xT chunks
        nc.scalar.dma_start(u2T[:], u2T_d.rearrange("(hb p) e -> p hb e", p=128))
        # big weight stream, grouped by output half so the first half of the
        # outputs can store while the second half still streams; v2 slots in
        # between (needed only by the lora matmuls near the end of phase A)
        for oc in range(OC):
            for hb in range(HB):
                nc.sync.dma_start(wT[:, hb, oc * 512:(oc + 1) * 512],
                                  wT_d[hb * 128:(hb + 1) * 128, oc * 512:(oc + 1) * 512])
            if oc == 0:
                nc.sync.dma_start(v2[:], v2_d[:])
        # PE warmup during the initial DMA dead-time: junk matmuls on a
        # locally-memset scratch tile (no DMA deps -> starts immediately) so
        # the PE clock (HAM) is at full rate when the real stream begins
        junk_sb = sb.tile([1, 256], f32, tag="junk_sb")
        nc.vector.memset(junk_sb[:], 0.0)
        warm_ps = ps.tile([1, 512], f32, tag="ps")
        N_WARM = 4
        for w in range(N_WARM):
            nc.tensor.matmul(warm_ps[:, 0:256], junk_sb[:1, 0:1], junk_sb[:],
                             start=(w == 0), stop=(w == N_WARM - 1),
                             skip_group_check=True)

        # PE stream part 1: gate logits (8 experts wide), tT
        gl_ps = ps.tile([8, TOK_PER_CORE], f32, tag="ps")
        for hb in range(HB):
            nc.tensor.matmul(gl_ps[:], gwT[:, hb, :], xT[:, hb, :],
                             start=(hb == 0), stop=(hb == HB - 1))
        t_ps = ps.tile([128, TOK_PER_CORE], f32, tag="ps")
        for hb in range(HB):
            nc.tensor.matmul(t_ps[:], u2T[:, hb, :], xT[:, hb, :],
                             start=(hb == 0), stop=(hb == HB - 1))

        # E = exp(logits) on ACT, concurrent with the tT matmuls on PE
        e_sb = sb.tile([8, TOK_PER_CORE], f32, tag="e_sb")
        nc.scalar.activation(e_sb[:], gl_ps[:], Exp)
        # sum over the 8 experts on POOL (keeps the PE stream clean); the
        # all-reduce leaves the sum in every partition, ready for the divide
        se_sb = sb.tile([8, TOK_PER_CORE], f32, tag="se_sb")
        nc.gpsimd.partition_all_reduce(se_sb[:], e_sb[:], channels=8,
                                       reduce_op=bass_isa.ReduceOp.add)
        # gate8 = E * (1/sum) on DVE
        rec_sb = sb.tile([8, TOK_PER_CORE], f32, tag="rec_sb")
        nc.vector.reciprocal(rec_sb[:], se_sb[:])
        gp8_sb = sb.tile([8, TOK_PER_CORE], f32r, tag="gp8_sb")
        with nc.allow_low_precision(reason="f32r rounding matches matmul precision"):
            nc.vector.tensor_tensor(gp8_sb[:], e_sb[:], rec_sb[:],
                                    mybir.AluOpType.mult)

        osb = ctx.enter_context(tc.tile_pool(name="osb", bufs=4))

        def acc_mm(accs, nb, oc, hb, start, stop):
            nc.tensor.matmul(accs[nb][:],
                             xT[:, hb, nb * 128:(nb + 1) * 128],
                             wT[:, hb, oc * 512:(oc + 1) * 512],
                             start=start, stop=stop, skip_group_check=True)

        def lora_mm(accs, nb, oc, start, stop=False):
            nc.tensor.matmul(accs[nb][:],
                             tp_sb[:, nb * 128:(nb + 1) * 128],
                             v2[:, oc * 512:(oc + 1) * 512],
                             start=start, stop=stop, skip_group_check=True)

        def store(accs, nb, oc):
            o_sb = osb.tile([128, 512], f32, tag="o", name=f"osb{oc}_{nb}")
            if nb % 2 == 0:
                nc.vector.tensor_copy(o_sb[:], accs[nb][:])
            else:
                nc.scalar.copy(o_sb[:], accs[nb][:])
            nc.scalar.dma_start(
                out_d[nb * 128:(nb + 1) * 128, oc * 512:(oc + 1) * 512], o_sb[:])

        # --- phase A (oc=0), hb-outer so PE tracks the wT chunk stream ---
        oc = 0
        accA = [ps.tile([128, 512], f32, tag="ps", name=f"accA{i}")
                for i in range(NB)]
        for nb in range(NB):
            acc_mm(accA, nb, oc, 0, True, False)
        for nb in range(NB):
            acc_mm(accA, nb, oc, 1, False, False)
        # expand to er=128 partitions: gp[er, n] = gp8[er//16, n]
        gp_ps = ps.tile([128, TOK_PER_CORE], f32, tag="ps")
        nc.tensor.matmul(gp_ps[:], sel[:], gp8_sb[:], start=True, stop=True)
        gp_sb = sb.tile([128, TOK_PER_CORE], f32, tag="gp_sb")
        nc.scalar.copy(gp_sb[:], gp_ps[:])
        tp_sb = sb.tile([128, TOK_PER_CORE], f32r, tag="tp_sb")
        with nc.allow_low_precision(reason="f32r rounding matches matmul precision"):
            nc.vector.tensor_tensor(tp_sb[:], t_ps[:], gp_sb[:],
                                    mybir.AluOpType.mult)
        for hb in range(2, HB):
            for nb in range(NB):
                acc_mm(accA, nb, oc, hb, False, False)
        for nb in range(NB):
            lora_mm(accA, nb, oc, False, stop=True)
        for nb in range(NB):
            store(accA, nb, oc)

        # --- phase B (oc=1) in two halves so the output DMAs stagger;
        # accumulators take the freed aux slots ---
        oc = 1
        accB = [ps.tile([128, 512], f32, tag="ps", name=f"accB{i}")
                for i in range(NB)]
        for lo, hi in ((0, 2), (2, 3), (3, NB)):
            for nb in range(lo, hi):
                lora_mm(accB, nb, oc, True)
            for hb in range(HB):
                for nb in range(lo, hi):
                    acc_mm(accB, nb, oc, hb, False, hb == HB - 1)
            for nb in range(lo, hi):
                if nb == NB - 1:
                    # final tile: half-width copies on both engines in
                    # parallel, two smaller stores to shorten the tail
                    o_sb = osb.tile([128, 512], f32, tag="o", name="osb_last")
                    nc.vector.tensor_copy(o_sb[:, 0:256], accB[nb][:, 0:256])
                    nc.scalar.copy(o_sb[:, 256:512], accB[nb][:, 256:512])
                    nc.scalar.dma_start(
                        out_d[nb * 128:(nb + 1) * 128, oc * 512:oc * 512 + 256],
                        o_sb[:, 0:256])
                    nc.scalar.dma_start(
                        out_d[nb * 128:(nb + 1) * 128, oc * 512 + 256:(oc + 1) * 512],
                        o_sb[:, 256:512])
                else:
                    store(accB, nb, oc)

    nc.compile()
    return nc


def _get_nc():
    if "nc" not in _CACHE:
        _CACHE["nc"] = _build()
    return _CACHE["nc"]


def _prep_in_maps(x, weight, gate_w, lora_U, lora_V):
    xt = np.ascontiguousarray(x.reshape(N_TOK, D_IN).T)          # (D_IN, N_TOK)
    wT = np.ascontiguousarray(weight.T)                          # (D_IN, D_OUT)
    u2T = np.ascontiguousarray(lora_U.reshape(ER, D_IN).T)       # (D_IN, ER)
    gwT = np.ascontiguousarray(gate_w.T)                         # (D_IN, E)
    consts = np.zeros((E, 768), dtype=np.float32)
    consts[:, 0] = 1.0                      # ones_col
    consts[0, 1:9] = 1.0                    # ones_row
    # lora scaling (1/R) folded into the expansion matrix
    consts[:, 9:9 + ER] = np.repeat(np.eye(E, dtype=np.float32), R, axis=0).T / R
    v2 = np.ascontiguousarray(lora_V.transpose(0, 2, 1).reshape(ER, D_OUT))
    in_maps = []
    for c in range(N_CORES):
        in_maps.append({
            "xT": np.ascontiguousarray(xt[:, c * TOK_PER_CORE:(c + 1) * TOK_PER_CORE]),
            "wT": wT,
            "u2T": u2T,
            "gwT": gwT,
            "v2": v2,
            "consts": consts,
        })
    return in_maps


def kernel(x, weight, gate_w, lora_U, lora_V):
    from concourse import bass_utils

    x = np.asarray(x, dtype=np.float32)
    weight = np.asarray(weight, dtype=np.float32)
    gate_w = np.asarray(gate_w, dtype=np.float32)
    lora_U = np.asarray(lora_U, dtype=np.float32)
    lora_V = np.asarray(lora_V, dtype=np.float32)

    nc = _get_nc()
    in_maps = _prep_in_maps(x, weight, gate_w, lora_U, lora_V)
    res = bass_utils.run_bass_kernel_spmd(nc, in_maps, core_ids=list(range(N_CORES)))
    out = np.concatenate([res.results[c]["out"] for c in range(N_CORES)], axis=0)
    return out.reshape(B, S, D_OUT)
